# revision 39
# baseline (speedup 1.0000x reference)
"""Trainium2 Bass kernel for additive (Bahdanau) attention.

  context[b] = sum_t softmax_t( v . tanh(We @ enc[b,t] + Wd @ dec[b] + bias) ) * enc[b,t]

Shapes (hardcoded): enc_out [64, 2048, 1024] f32, dec_state [64, 1024] f32,
W_weight [1024, 2048], W_bias [1024], v_weight [1, 1024].  Output [64, 1024].

Sharding: data-parallel over batch across 8 NeuronCores (8 batches/core).

Design (v2, fp8 DoubleRow):
- Host prep: enc is cast twice — bf16 in [b, tl, i, e] layout (ctx matmul
  stream) and fp8-e4m3 in pre-transposed [b, el, i, j, tl] layout (proj
  stationary), so no on-device transposes at all.  We^T is cast to fp8 in
  [el, j, d] pair layout; z = Wd@dec + bias is split z8 + zr8 (fp8 residual
  pair) and replicated across 128 partitions.
- proj = X @ (32*We^T) runs as fp8 MatmulPerfMode.DoubleRow (2 K-tiles per
  instruction; on HW each N=512 matmul costs ~259ns regardless of K, so DR
  halves the pass count: 8 matmuls/tile).  The x32 weight scaling keeps the
  small We values out of e4m3's subnormal range (quantization error there
  otherwise dominates) and is undone exactly by tanh's scale=1/32.
- z bias (scaled x32, bf16) is added to the PSUM output on DVE (Pool/GPSIMD
  cannot access PSUM), keeping the z-add off the bottleneck PE.
- Epilogue per 128-row tile: DVE add (PSUM f32 + zrep -> SBUF bf16); ACT
  tanh(scale=1/32); Pool scalar_tensor_tensor (x v) reduces to scores; ACT
  exp emits bf16 softmax weights; PE accumulates ctx += p^T @ X in bf16.
  Softmax needs no max-subtraction (|scores| <= sum|v| <= 32).
- One global software pipeline over 128 row-tiles; DMA batched 4 tiles per
  instruction (all runs >= 1KB contiguous).
"""

import sys

sys.path.insert(0, "/opt/trn_rl_repo")

from contextlib import ExitStack

import ml_dtypes
import numpy as np

import concourse.bass as bass
import concourse.tile as tile
from concourse import bacc, mybir
from concourse.bass_utils import run_bass_kernel_spmd

F32 = mybir.dt.float32
BF16 = mybir.dt.bfloat16
FP8 = mybir.dt.float8e4
NP_FP8 = ml_dtypes.float8_e4m3
NP_BF16 = ml_dtypes.bfloat16
DR = mybir.MatmulPerfMode.DoubleRow

B, T, E, D = 64, 2048, 1024, 1024
CORES = 8
BL = B // CORES           # batches per core (8)
P = 128                   # partitions
TT = T // P               # t-tiles per batch (16)
ET = E // P               # e-blocks per row-tile (8)
QUAD = 4                  # t-tiles fetched per DMA instruction
CTX_LAG = 4               # t-tiles of lag before emitting ctx matmuls
EXP_LAG = 3               # t-tiles of lag before emitting exp (vs proj)
END_LAG = 2               # extra t-tiles before emitting batch-end chain
PREFETCH_QUADS = 3
WSCALE = 32.0             # fp8 subnormal-avoidance scale on We^T and z


def _build_kernel(bl=BL, t_tiles=TT):
    nc = bacc.Bacc(
        "TRN2",
        target_bir_lowering=False,
        debug=False,
        num_devices=CORES,
    )

    # [b, tl, i, e]: x16[b, tl, i, :] = enc[b, i*128+tl, :] in bf16
    x16 = nc.declare_dram_parameter("x16", [bl, P, t_tiles, E], BF16, isOutput=False)
    # [b, el, i, j, tl]: xt8[b, el, i, j, tl] = enc[b, i*128+tl, j*128+el] in fp8
    xt8 = nc.declare_dram_parameter("xt8", [bl, P, t_tiles, ET, P], FP8, isOutput=False)
    # [el, j, d]: wet8[el, j, d] = 32 * We[d, j*128+el] in fp8
    wet8 = nc.declare_dram_parameter("wet8", [P, ET, D], FP8, isOutput=False)
    # [k, b, d]: 32 * z[b, d] in bf16, replicated over k partitions
    zrep16 = nc.declare_dram_parameter("zrep16", [P, bl, D], BF16, isOutput=False)
    v16 = nc.declare_dram_parameter("v16", [P, D], BF16, isOutput=False)
    onesc = nc.declare_dram_parameter("onesc", [P, 1], F32, isOutput=False)
    out = nc.declare_dram_parameter("ctx_out", [bl, E], F32, isOutput=True)

    n_quads_total = bl * t_tiles // QUAD

    with tile.TileContext(nc) as tc, ExitStack() as ctx:
        const = ctx.enter_context(tc.tile_pool(name="const", bufs=1))
        xq_pool = ctx.enter_context(tc.tile_pool(name="xq", bufs=4))
        xtq_pool = ctx.enter_context(tc.tile_pool(name="xtq", bufs=4))
        epool = ctx.enter_context(tc.tile_pool(name="e", bufs=4))
        small = ctx.enter_context(tc.tile_pool(name="small", bufs=2))

        ps_proj = ctx.enter_context(tc.tile_pool(name="ps_proj", bufs=5, space="PSUM"))
        ps_ctx = ctx.enter_context(tc.tile_pool(name="ps_ctx", bufs=2, space="PSUM"))
        ps_misc = ctx.enter_context(tc.tile_pool(name="ps_misc", bufs=1, space="PSUM"))

        # ---- resident constants.  Ordered so proj(0)'s inputs land first.
        xq_tiles = {}
        xtq_tiles = {}

        def fetch_quad(q, skip_xq=False):
            b, qi = divmod(q, t_tiles // QUAD)
            if not skip_xq:
                xq = xq_pool.tile([P, QUAD, E], BF16, tag="xq")
                nc.sync.dma_start(xq[:], x16[b, :, QUAD * qi : QUAD * (qi + 1), :])
                xq_tiles[q] = xq
            xtq = xtq_pool.tile([P, QUAD, ET, P], FP8, tag="xtq")
            nc.sync.dma_start(xtq[:], xt8[b, :, QUAD * qi : QUAD * (qi + 1), :, :])
            xtq_tiles[q] = xtq

        fetch_quad(0, skip_xq=True)
        wet_t = []
        for pr in range(ET // 2):
            wt = const.tile([P, 2, D], FP8, name=f"wet{pr}")
            nc.sync.dma_start(wt[:], wet8[:, 2 * pr : 2 * pr + 2, :])
            wet_t.append(wt)
        z_t = [const.tile([P, D], BF16, name=f"z{b}") for b in range(bl)]
        nc.sync.dma_start(z_t[0][:], zrep16[:, 0])
        v_sb = const.tile([P, D], BF16)
        nc.sync.dma_start(v_sb[:], v16[:])
        xq0 = xq_pool.tile([P, QUAD, E], BF16, tag="xq")
        nc.sync.dma_start(xq0[:], x16[0, :, 0:QUAD, :])
        xq_tiles[0] = xq0
        onesc_sb = const.tile([P, 1], F32)
        nc.sync.dma_start(onesc_sb[:], onesc[:])

        # ---- per-batch state ------------------------------------------------
        total = bl * t_tiles
        state = {}

        def get_state(b):
            if b not in state:
                state[b] = dict(
                    s_all=small.tile([P, t_tiles], F32, tag="s", name=f"s_all_{b}"),
                    s1_all=small.tile([P, t_tiles], F32, tag="s1", name=f"s1_all_{b}"),
                    p_all=small.tile([P, t_tiles], BF16, tag="p", name=f"p_all_{b}"),
                    ctx0=ps_ctx.tile([1, 512], F32, tag="ps_ctx", name=f"ctx0_{b}"),
                    ctx1=ps_ctx.tile([1, 512], F32, tag="ps_ctx", name=f"ctx1_{b}"),
                    proj_ps=[None] * t_tiles,
                    e_sbs=[None] * t_tiles,
                )
            return state[b]

        def emit_proj(b, i):
            # proj[t, d] = sum_e x[t, e] * 32*WeT[e, d], fp8 DoubleRow
            st = get_state(b)
            k = b * t_tiles + i
            q, qi = divmod(k, QUAD)
            xtq = xtq_tiles[q]
            pj = [
                ps_proj.tile([P, 512], F32, tag="ps_proj", name=f"pj{h}_{b}_{i}")
                for h in range(2)
            ]
            st["proj_ps"][i] = pj
            for pr in range(ET // 2):
                lhs = xtq[:, qi, 2 * pr : 2 * pr + 2, :]
                for h in range(2):
                    sl = slice(h * 512, (h + 1) * 512)
                    nc.tensor.matmul(
                        pj[h][:], lhs, wet_t[pr][:, :, sl],
                        start=(pr == 0), stop=(pr == ET // 2 - 1), perf_mode=DR,
                    )


        H = 384  # DVE v-dot on [0:H); Pool mult + ACT reduce on [H:D)

        def emit_epilogue(b, i):
            # energy = tanh((proj + 32z)/32); s = sum_d energy*v, split per
            # PSUM half so each stage starts as soon as its half is ready
            st = get_state(b)
            pj = st["proj_ps"][i]
            st["proj_ps"][i] = None
            e_sb = epool.tile([P, D], BF16, tag="e")
            st["e_sbs"][i] = e_sb
            for h in range(2):
                sl = slice(h * 512, (h + 1) * 512)
                nc.vector.tensor_add(e_sb[:, sl], pj[h][:], z_t[b][:, sl])
            nc.scalar.activation(
                e_sb[:], e_sb[:], mybir.ActivationFunctionType.Tanh,
                scale=1.0 / WSCALE,
            )
            nc.vector.scalar_tensor_tensor(
                out=e_sb[:, 0:H],
                in0=e_sb[:, 0:H],
                scalar=1.0,
                in1=v_sb[:, 0:H],
                op0=mybir.AluOpType.mult,
                op1=mybir.AluOpType.mult,
                accum_out=st["s_all"][:, i : i + 1],
            )
            nc.gpsimd.tensor_tensor(
                e_sb[:, H:D], e_sb[:, H:D], v_sb[:, H:D], mybir.AluOpType.mult
            )

        def emit_reduce(b, i):
            # s1 = sum of the Pool-multiplied half; deferred one step so the
            # ACT queue never waits on the Pool round-trip
            st = get_state(b)
            e_sb = st["e_sbs"][i]
            st["e_sbs"][i] = None
            nc.scalar.activation(
                e_sb[:, H:D], e_sb[:, H:D], mybir.ActivationFunctionType.Copy,
                accum_out=st["s1_all"][:, i : i + 1],
            )

        def emit_exp(b, i):
            # p = exp(s0 + s1); deferred so this ACT instr never blocks a tanh
            st = get_state(b)
            nc.scalar.activation(
                st["p_all"][:, i : i + 1],
                st["s_all"][:, i : i + 1],
                mybir.ActivationFunctionType.Exp,
                bias=st["s1_all"][:, i : i + 1],
            )

        def emit_ctx_half(b, i, h):
            # ctx_unnorm += p^T @ X  (contraction over the 128 t-rows), bf16
            st = get_state(b)
            k = b * t_tiles + i
            q, qi = divmod(k, QUAD)
            xq = xq_tiles[q]
            p_col = st["p_all"][:, i : i + 1]
            nc.tensor.matmul(
                st["ctx0" if h == 0 else "ctx1"][:], p_col,
                xq[:, qi, h * 512 : (h + 1) * 512],
                start=(i == 0), stop=(i == t_tiles - 1),
            )

        def emit_ctx(b, i):
            emit_ctx_half(b, i, 0)
            emit_ctx_half(b, i, 1)

        def emit_batch_end(b):
            # l = sum_t exp(s_t); ctx = ctx_unnorm / l.  The partition sum is
            # done via DMA-transpose + DVE reduce to keep it off the PE/PSUM.
            st = state.pop(b)
            l_part = small.tile([P, 1], F32, tag="lp")
            nc.vector.tensor_reduce(
                l_part[:], st["p_all"][:],
                axis=mybir.AxisListType.X, op=mybir.AluOpType.add,
            )
            l_ps = ps_misc.tile([1, 1], F32, tag="ps_misc")
            nc.tensor.matmul(l_ps[:], l_part[:], onesc_sb[:])
            linv = small.tile([1, 1], F32, tag="linv")
            nc.vector.reciprocal(linv[:], l_ps[:])
            ctx_row = small.tile([1, E], F32, tag="ctxrow")
            nc.scalar.activation(
                ctx_row[:, 0:512], st["ctx0"][:],
                mybir.ActivationFunctionType.Copy, scale=linv[:],
            )
            nc.scalar.activation(
                ctx_row[:, 512:E], st["ctx1"][:],
                mybir.ActivationFunctionType.Copy, scale=linv[:],
            )
            nc.sync.dma_start(out[b : b + 1, :], ctx_row[:])

        # ---- main software pipeline over all (batch, t-tile) ----------------
        for k in range(total + CTX_LAG + END_LAG):
            if k < total:
                emit_proj(*divmod(k, t_tiles))
            if k == 0:
                for q in range(1, PREFETCH_QUADS):
                    fetch_quad(q)
            if k % QUAD == 0:
                qf = k // QUAD + PREFETCH_QUADS
                if qf < n_quads_total:
                    fetch_quad(qf)
            if k % t_tiles == 8 and k // t_tiles + 1 < bl:
                b_next = k // t_tiles + 1
                nc.sync.dma_start(z_t[b_next][:], zrep16[:, b_next])
            kc = k - CTX_LAG
            if 0 <= kc < total and kc % QUAD == QUAD - 1:
                for kk in range(kc - QUAD + 1, kc + 1):
                    emit_ctx(*divmod(kk, t_tiles))
            if 0 <= k - 1 < total:
                emit_epilogue(*divmod(k - 1, t_tiles))
            if 0 <= k - 2 < total:
                emit_reduce(*divmod(k - 2, t_tiles))
            if 0 <= k - EXP_LAG < total:
                emit_exp(*divmod(k - EXP_LAG, t_tiles))
            kb = k - CTX_LAG - END_LAG
            if 0 <= kb < total and kb % t_tiles == t_tiles - 1:
                emit_batch_end(kb // t_tiles)

    nc.compile()
    return nc


def _prep_inputs(enc_out, dec_state, W_weight, W_bias, v_weight, bl=BL):
    """Host-side layout/dtype prep + per-core slicing."""
    enc_out = np.ascontiguousarray(enc_out, dtype=np.float32)
    dec_state = np.ascontiguousarray(dec_state, dtype=np.float32)
    W = np.asarray(W_weight, dtype=np.float32)

    # x16: [B, tl, i, e] bf16
    x16_h = np.ascontiguousarray(
        enc_out.reshape(B, TT, P, E).transpose(0, 2, 1, 3).astype(NP_BF16)
    )
    # xt8: [B, el, i, j, tl] fp8
    enc8 = enc_out.astype(NP_FP8)
    xt8_h = np.ascontiguousarray(
        enc8.reshape(B, TT, P, ET, P).transpose(0, 4, 1, 3, 2)
    )
    # wet8: [el, j, d], scaled by WSCALE to avoid e4m3 subnormals
    wet8_h = np.ascontiguousarray(
        (WSCALE * W[:, :E].T).astype(NP_FP8).reshape(ET, P, D).transpose(1, 0, 2)
    )
    # z = Wd @ dec + bias, scaled by WSCALE, bf16, replicated over k
    z_all = dec_state @ W[:, E:].T + np.asarray(W_bias, dtype=np.float32)  # [B, D]
    z16 = (WSCALE * z_all).astype(NP_BF16)
    v16_h = np.ascontiguousarray(
        np.broadcast_to(np.asarray(v_weight).astype(NP_BF16).reshape(1, D), (P, D))
    )
    onesc_h = np.ones((P, 1), dtype=np.float32)

    in_maps = []
    for c in range(CORES):
        sl = slice(c * bl, (c + 1) * bl)
        zrep_h = np.ascontiguousarray(np.broadcast_to(z16[None, sl], (P, bl, D)))
        in_maps.append(
            {
                "x16": x16_h[sl],
                "xt8": xt8_h[sl],
                "wet8": wet8_h,
                "zrep16": zrep_h,
                "v16": v16_h,
                "onesc": onesc_h,
            }
        )
    return in_maps


_NC_CACHE = {}


def _get_nc():
    if "nc" not in _NC_CACHE:
        _NC_CACHE["nc"] = _build_kernel()
    return _NC_CACHE["nc"]


def _run(inputs, trace=False, tmpdir=None):
    nc = _get_nc()
    in_maps = _prep_inputs(
        inputs["enc_out"],
        inputs["dec_state"],
        inputs["W_weight"],
        inputs["W_bias"],
        inputs["v_weight"],
    )
    res = run_bass_kernel_spmd(
        nc, in_maps, list(range(CORES)), trace=trace, tmpdir=tmpdir
    )
    out = np.concatenate(
        [np.asarray(res.results[c]["ctx_out"]) for c in range(CORES)], axis=0
    )
    return out.astype(np.float32, copy=False), res


def kernel(**inputs):
    out, _ = _run(inputs, trace=False)
    return out


if __name__ == "__main__":
    pass


# revision 40
# speedup vs baseline: 1.1909x; 1.1909x over previous
"""Trainium2 Bass kernel for additive (Bahdanau) attention.

  context[b] = sum_t softmax_t( v . tanh(We @ enc[b,t] + Wd @ dec[b] + bias) ) * enc[b,t]

Shapes (hardcoded): enc_out [64, 2048, 1024] f32, dec_state [64, 1024] f32,
W_weight [1024, 2048], W_bias [1024], v_weight [1, 1024].  Output [64, 1024].

Sharding: data-parallel over batch across 8 NeuronCores (8 batches/core).

Design (v2, fp8 DoubleRow):
- Host prep: enc is cast twice — bf16 in [b, tl, i, e] layout (ctx matmul
  stream) and fp8-e4m3 in pre-transposed [b, el, i, j, tl] layout (proj
  stationary), so no on-device transposes at all.  We^T is cast to fp8 in
  [el, j, d] pair layout; z = Wd@dec + bias is split z8 + zr8 (fp8 residual
  pair) and replicated across 128 partitions.
- proj = X @ (32*We^T) runs as fp8 MatmulPerfMode.DoubleRow (2 K-tiles per
  instruction; on HW each N=512 matmul costs ~259ns regardless of K, so DR
  halves the pass count: 8 matmuls/tile).  The x32 weight scaling keeps the
  small We values out of e4m3's subnormal range (quantization error there
  otherwise dominates) and is undone exactly by tanh's scale=1/32.
- z bias (scaled x32, bf16) is added to the PSUM output on DVE (Pool/GPSIMD
  cannot access PSUM), keeping the z-add off the bottleneck PE.
- Epilogue per 128-row tile: DVE add (PSUM f32 + zrep -> SBUF bf16); ACT
  tanh(scale=1/32); Pool scalar_tensor_tensor (x v) reduces to scores; ACT
  exp emits bf16 softmax weights; PE accumulates ctx += p^T @ X in bf16.
  Softmax needs no max-subtraction (|scores| <= sum|v| <= 32).
- One global software pipeline over 128 row-tiles; DMA batched 4 tiles per
  instruction (all runs >= 1KB contiguous).
"""

import sys

sys.path.insert(0, "/opt/trn_rl_repo")

from contextlib import ExitStack

import ml_dtypes
import numpy as np

import concourse.bass as bass
import concourse.tile as tile
from concourse import bacc, mybir
from concourse.bass_utils import run_bass_kernel_spmd

F32 = mybir.dt.float32
BF16 = mybir.dt.bfloat16
FP8 = mybir.dt.float8e4
NP_FP8 = ml_dtypes.float8_e4m3
NP_BF16 = ml_dtypes.bfloat16
DR = mybir.MatmulPerfMode.DoubleRow

B, T, E, D = 64, 2048, 1024, 1024
CORES = 8
BL = B // CORES           # batches per core (8)
P = 128                   # partitions
TT = T // P               # t-tiles per batch (16)
ET = E // P               # e-blocks per row-tile (8)
QUAD = 4                  # t-tiles fetched per DMA instruction
CTX_LAG = 4               # t-tiles of lag before emitting ctx matmuls
EXP_LAG = 3               # t-tiles of lag before emitting exp (vs proj)
END_LAG = 2               # extra t-tiles before emitting batch-end chain
PREFETCH_QUADS = 3
WSCALE = 32.0             # fp8 subnormal-avoidance scale on We^T and z


def _build_kernel(bl=BL, t_tiles=TT):
    nc = bacc.Bacc(
        "TRN2",
        target_bir_lowering=False,
        debug=False,
        num_devices=CORES,
    )

    # [b, tl, i, e]: x16[b, tl, i, :] = enc[b, i*128+tl, :] in bf16
    x16 = nc.declare_dram_parameter("x16", [bl, P, t_tiles, E], BF16, isOutput=False)
    # [b, el, i, j, tl]: xt8[b, el, i, j, tl] = enc[b, i*128+tl, j*128+el] in fp8
    xt8 = nc.declare_dram_parameter("xt8", [bl, P, t_tiles, ET, P], FP8, isOutput=False)
    # [el, j, d]: wet8[el, j, d] = 32 * We[d, j*128+el] in fp8
    wet8 = nc.declare_dram_parameter("wet8", [P, ET, D], FP8, isOutput=False)
    # [k, b, d]: 32 * z[b, d] in bf16, replicated over k partitions
    zrep16 = nc.declare_dram_parameter("zrep16", [P, bl, D], BF16, isOutput=False)
    v16 = nc.declare_dram_parameter("v16", [P, D], BF16, isOutput=False)
    onesc = nc.declare_dram_parameter("onesc", [P, 1], F32, isOutput=False)
    out = nc.declare_dram_parameter("ctx_out", [bl, E], F32, isOutput=True)

    n_quads_total = bl * t_tiles // QUAD

    with tile.TileContext(nc) as tc, ExitStack() as ctx:
        const = ctx.enter_context(tc.tile_pool(name="const", bufs=1))
        xq_pool = ctx.enter_context(tc.tile_pool(name="xq", bufs=4))
        xtq_pool = ctx.enter_context(tc.tile_pool(name="xtq", bufs=4))
        epool = ctx.enter_context(tc.tile_pool(name="e", bufs=4))
        small = ctx.enter_context(tc.tile_pool(name="small", bufs=2))

        ps_proj = ctx.enter_context(tc.tile_pool(name="ps_proj", bufs=5, space="PSUM"))
        ps_ctx = ctx.enter_context(tc.tile_pool(name="ps_ctx", bufs=2, space="PSUM"))
        ps_misc = ctx.enter_context(tc.tile_pool(name="ps_misc", bufs=1, space="PSUM"))

        # ---- resident constants.  Ordered so proj(0)'s inputs land first.
        xq_tiles = {}
        xtq_tiles = {}

        def fetch_quad(q, skip_xq=False):
            b, qi = divmod(q, t_tiles // QUAD)
            if not skip_xq:
                xq = xq_pool.tile([P, QUAD, E], BF16, tag="xq")
                nc.sync.dma_start(xq[:], x16[b, :, QUAD * qi : QUAD * (qi + 1), :])
                xq_tiles[q] = xq
            xtq = xtq_pool.tile([P, QUAD, ET, P], FP8, tag="xtq")
            nc.sync.dma_start(xtq[:], xt8[b, :, QUAD * qi : QUAD * (qi + 1), :, :])
            xtq_tiles[q] = xtq

        fetch_quad(0, skip_xq=True)
        wet_t = []
        for pr in range(ET // 2):
            wt = const.tile([P, 2, D], FP8, name=f"wet{pr}")
            nc.sync.dma_start(wt[:], wet8[:, 2 * pr : 2 * pr + 2, :])
            wet_t.append(wt)
        z_t = [const.tile([P, D], BF16, name=f"z{b}") for b in range(bl)]
        nc.sync.dma_start(z_t[0][:], zrep16[:, 0])
        v_sb = const.tile([P, D], BF16)
        nc.sync.dma_start(v_sb[:], v16[:])
        xq0 = xq_pool.tile([P, QUAD, E], BF16, tag="xq")
        nc.sync.dma_start(xq0[:], x16[0, :, 0:QUAD, :])
        xq_tiles[0] = xq0
        onesc_sb = const.tile([P, 1], F32)
        nc.sync.dma_start(onesc_sb[:], onesc[:])

        # ---- per-batch state ------------------------------------------------
        total = bl * t_tiles
        state = {}

        def get_state(b):
            if b not in state:
                state[b] = dict(
                    s_all=small.tile([P, t_tiles], F32, tag="s", name=f"s_all_{b}"),
                    s1_all=small.tile([P, t_tiles], F32, tag="s1", name=f"s1_all_{b}"),
                    p_all=small.tile([P, t_tiles], BF16, tag="p", name=f"p_all_{b}"),
                    ctx0=ps_ctx.tile([1, 512], F32, tag="ps_ctx", name=f"ctx0_{b}"),
                    ctx1=ps_ctx.tile([1, 512], F32, tag="ps_ctx", name=f"ctx1_{b}"),
                    proj_ps=[None] * t_tiles,
                    e_sbs=[None] * t_tiles,
                )
            return state[b]

        def emit_proj(b, i):
            # proj[t, d] = sum_e x[t, e] * 32*WeT[e, d], fp8 DoubleRow
            st = get_state(b)
            k = b * t_tiles + i
            q, qi = divmod(k, QUAD)
            xtq = xtq_tiles[q]
            pj = [
                ps_proj.tile([P, 512], F32, tag="ps_proj", name=f"pj{h}_{b}_{i}")
                for h in range(2)
            ]
            st["proj_ps"][i] = pj
            for pr in range(ET // 2):
                lhs = xtq[:, qi, 2 * pr : 2 * pr + 2, :]
                for h in range(2):
                    sl = slice(h * 512, (h + 1) * 512)
                    nc.tensor.matmul(
                        pj[h][:], lhs, wet_t[pr][:, :, sl],
                        start=(pr == 0), stop=(pr == ET // 2 - 1), perf_mode=DR,
                    )


        H = 384  # DVE v-dot on [0:H); Pool mult + ACT reduce on [H:D)

        def emit_epilogue(b, i):
            # energy = tanh((proj + 32z)/32); s = sum_d energy*v, split per
            # PSUM half so each stage starts as soon as its half is ready
            st = get_state(b)
            pj = st["proj_ps"][i]
            st["proj_ps"][i] = None
            e_sb = epool.tile([P, D], BF16, tag="e")
            st["e_sbs"][i] = e_sb
            for h in range(2):
                sl = slice(h * 512, (h + 1) * 512)
                nc.vector.tensor_add(e_sb[:, sl], pj[h][:], z_t[b][:, sl])
            nc.scalar.activation(
                e_sb[:], e_sb[:], mybir.ActivationFunctionType.Tanh,
                scale=1.0 / WSCALE,
            )
            nc.vector.scalar_tensor_tensor(
                out=e_sb[:, 0:H],
                in0=e_sb[:, 0:H],
                scalar=1.0,
                in1=v_sb[:, 0:H],
                op0=mybir.AluOpType.mult,
                op1=mybir.AluOpType.mult,
                accum_out=st["s_all"][:, i : i + 1],
            )
            nc.gpsimd.tensor_tensor(
                e_sb[:, H:D], e_sb[:, H:D], v_sb[:, H:D], mybir.AluOpType.mult
            )

        def emit_reduce(b, i):
            # s1 = sum of the Pool-multiplied half; deferred one step so the
            # ACT queue never waits on the Pool round-trip
            st = get_state(b)
            e_sb = st["e_sbs"][i]
            st["e_sbs"][i] = None
            nc.scalar.activation(
                e_sb[:, H:D], e_sb[:, H:D], mybir.ActivationFunctionType.Copy,
                accum_out=st["s1_all"][:, i : i + 1],
            )

        def emit_exp(b, i):
            # p = exp(s0 + s1); deferred so this ACT instr never blocks a tanh
            st = get_state(b)
            nc.scalar.activation(
                st["p_all"][:, i : i + 1],
                st["s_all"][:, i : i + 1],
                mybir.ActivationFunctionType.Exp,
                bias=st["s1_all"][:, i : i + 1],
            )

        def emit_ctx_half(b, i, h):
            # ctx_unnorm += p^T @ X  (contraction over the 128 t-rows), bf16
            st = get_state(b)
            k = b * t_tiles + i
            q, qi = divmod(k, QUAD)
            xq = xq_tiles[q]
            p_col = st["p_all"][:, i : i + 1]
            nc.tensor.matmul(
                st["ctx0" if h == 0 else "ctx1"][:], p_col,
                xq[:, qi, h * 512 : (h + 1) * 512],
                start=(i == 0), stop=(i == t_tiles - 1),
            )

        def emit_ctx(b, i):
            emit_ctx_half(b, i, 0)
            emit_ctx_half(b, i, 1)

        def emit_batch_end(b):
            # l = sum_t exp(s_t); ctx = ctx_unnorm / l.  The partition sum is
            # done via DMA-transpose + DVE reduce to keep it off the PE/PSUM.
            st = state.pop(b)
            l_part = small.tile([P, 1], F32, tag="lp")
            nc.vector.tensor_reduce(
                l_part[:], st["p_all"][:],
                axis=mybir.AxisListType.X, op=mybir.AluOpType.add,
            )
            l_ps = ps_misc.tile([1, 1], F32, tag="ps_misc")
            nc.tensor.matmul(l_ps[:], l_part[:], onesc_sb[:])
            linv = small.tile([1, 1], F32, tag="linv")
            nc.vector.reciprocal(linv[:], l_ps[:])
            ctx_row = small.tile([1, E], F32, tag="ctxrow")
            nc.scalar.activation(
                ctx_row[:, 0:512], st["ctx0"][:],
                mybir.ActivationFunctionType.Copy, scale=linv[:],
            )
            nc.scalar.activation(
                ctx_row[:, 512:E], st["ctx1"][:],
                mybir.ActivationFunctionType.Copy, scale=linv[:],
            )
            nc.sync.dma_start(out[b : b + 1, :], ctx_row[:])

        # ---- main software pipeline over all (batch, t-tile) ----------------
        for k in range(total + CTX_LAG + END_LAG):
            if k < total:
                emit_proj(*divmod(k, t_tiles))
            if k == 0:
                for q in range(1, PREFETCH_QUADS):
                    fetch_quad(q)
            if k % QUAD == 0:
                qf = k // QUAD + PREFETCH_QUADS
                if qf < n_quads_total:
                    fetch_quad(qf)
            if k % t_tiles == 8 and k // t_tiles + 1 < bl:
                b_next = k // t_tiles + 1
                nc.sync.dma_start(z_t[b_next][:], zrep16[:, b_next])
            if 0 <= k - CTX_LAG < total:
                emit_ctx(*divmod(k - CTX_LAG, t_tiles))
            if 0 <= k - 1 < total:
                emit_epilogue(*divmod(k - 1, t_tiles))
            if 0 <= k - 2 < total:
                emit_reduce(*divmod(k - 2, t_tiles))
            if 0 <= k - EXP_LAG < total:
                emit_exp(*divmod(k - EXP_LAG, t_tiles))
            kb = k - CTX_LAG - END_LAG
            if 0 <= kb < total and kb % t_tiles == t_tiles - 1:
                emit_batch_end(kb // t_tiles)

    nc.compile()
    return nc


def _prep_inputs(enc_out, dec_state, W_weight, W_bias, v_weight, bl=BL):
    """Host-side layout/dtype prep + per-core slicing."""
    enc_out = np.ascontiguousarray(enc_out, dtype=np.float32)
    dec_state = np.ascontiguousarray(dec_state, dtype=np.float32)
    W = np.asarray(W_weight, dtype=np.float32)

    # x16: [B, tl, i, e] bf16
    x16_h = np.ascontiguousarray(
        enc_out.reshape(B, TT, P, E).transpose(0, 2, 1, 3).astype(NP_BF16)
    )
    # xt8: [B, el, i, j, tl] fp8
    enc8 = enc_out.astype(NP_FP8)
    xt8_h = np.ascontiguousarray(
        enc8.reshape(B, TT, P, ET, P).transpose(0, 4, 1, 3, 2)
    )
    # wet8: [el, j, d], scaled by WSCALE to avoid e4m3 subnormals
    wet8_h = np.ascontiguousarray(
        (WSCALE * W[:, :E].T).astype(NP_FP8).reshape(ET, P, D).transpose(1, 0, 2)
    )
    # z = Wd @ dec + bias, scaled by WSCALE, bf16, replicated over k
    z_all = dec_state @ W[:, E:].T + np.asarray(W_bias, dtype=np.float32)  # [B, D]
    z16 = (WSCALE * z_all).astype(NP_BF16)
    v16_h = np.ascontiguousarray(
        np.broadcast_to(np.asarray(v_weight).astype(NP_BF16).reshape(1, D), (P, D))
    )
    onesc_h = np.ones((P, 1), dtype=np.float32)

    in_maps = []
    for c in range(CORES):
        sl = slice(c * bl, (c + 1) * bl)
        zrep_h = np.ascontiguousarray(np.broadcast_to(z16[None, sl], (P, bl, D)))
        in_maps.append(
            {
                "x16": x16_h[sl],
                "xt8": xt8_h[sl],
                "wet8": wet8_h,
                "zrep16": zrep_h,
                "v16": v16_h,
                "onesc": onesc_h,
            }
        )
    return in_maps


_NC_CACHE = {}


def _get_nc():
    if "nc" not in _NC_CACHE:
        _NC_CACHE["nc"] = _build_kernel()
    return _NC_CACHE["nc"]


def _run(inputs, trace=False, tmpdir=None):
    nc = _get_nc()
    in_maps = _prep_inputs(
        inputs["enc_out"],
        inputs["dec_state"],
        inputs["W_weight"],
        inputs["W_bias"],
        inputs["v_weight"],
    )
    res = run_bass_kernel_spmd(
        nc, in_maps, list(range(CORES)), trace=trace, tmpdir=tmpdir
    )
    out = np.concatenate(
        [np.asarray(res.results[c]["ctx_out"]) for c in range(CORES)], axis=0
    )
    return out.astype(np.float32, copy=False), res


def kernel(**inputs):
    out, _ = _run(inputs, trace=False)
    return out


if __name__ == "__main__":
    pass
